# revision 1
# baseline (speedup 1.0000x reference)
"""Distributed multi-head attention kernel for 8 Trainium2 NeuronCores.

Problem: y = softmax((x Wq^T)(x Wk^T)^T / sqrt(D)) (x Wv^T) Wo^T + bo
with B=4, T=2048, C=1280, H=20, D=64, float32 I/O.

Sharding (sequence parallel, rank independent):
  Each core owns a T/8 token slice of all 4 batches (1024 tokens).
  It computes Q/K/V projections for its tokens, AllGathers K^T and V
  (bf16, chunked per batch pair so comm overlaps compute), runs full
  attention for its queries over the gathered keys/values of the
  matching batch, and applies the output projection for its tokens.
  The host reassembles the T axis.

Attention runs in a transposed "S_T[k, q]" layout so the softmax
denominator falls out of the same matmul that computes P@V: V is
stored padded per head as [.., 65] with a constant-1 column, so psum
row 64 of the P@V accumulation is sum_k P. This avoids partition-axis
reductions entirely. exp() runs on ScalarE directly out of PSUM in
[128, 4*TS] chunks; P@V is interleaved per chunk so the PE keeps
working while ScalarE drains. Q-projection for batches 2-3 is emitted
after batch-0 attention as PE gap filler.

Compute dtype is bf16 (fp32 matmul is 4x slower on the PE array);
accumulation is fp32 in PSUM. I/O stays fp32.
"""

import os
import sys
import types

import numpy as np
import ml_dtypes

import concourse.bass as bass
import concourse.mybir as mybir
import concourse.tile as tile
from concourse import bacc
from concourse.bass_utils import run_bass_kernel_spmd

N_CORES = 8
C = 1280
H = 20
D = 64
B = 4
CT = C // 128  # 10 c-tiles
HP5 = H * 65   # padded V row width (64 dims + ones column per head)
BF = mybir.dt.bfloat16
F32 = mybir.dt.float32
SCALE = 1.0 / (D ** 0.5)

LAST_EXEC_TIME_NS = None
_BUILD_CACHE = {}


def _install_ntff_hook():
    """The trimmed antenv package lacks axon_hooks; register the NTFF
    profile hook by hand so trace=True can time the NEFF on silicon.
    Safe no-op if anything is missing."""
    if "antenv.axon_hooks" in sys.modules:
        return
    try:
        from trn_agent_boot.trn_boot import _ntff_profile_via_ctypes

        hook = _ntff_profile_via_ctypes("/opt/axon/libaxon_pjrt.so")
        mod = types.ModuleType("antenv.axon_hooks")
        mod.get_axon_ntff_profile_hook = lambda: hook
        mod.set_axon_ntff_profile_hook = lambda h: None
        sys.modules["antenv.axon_hooks"] = mod
        import antenv

        antenv.axon_hooks = mod
    except Exception:
        pass


def _chunks(total, step):
    out = []
    o = 0
    while o < total:
        out.append((o, min(step, total - o)))
        o += step
    return out


def build(T):
    """Build the SPMD Bass graph for full (unsharded) sequence length T."""
    TS = T // N_CORES          # tokens per batch per core
    TOK = B * TS               # tokens per core
    KT = T // 128              # 128-wide key tiles per batch
    assert TS % 128 == 0, "key tiles must not cross rank chunks"
    assert TS <= 512, "q tile must fit one matmul moving operand"
    assert KT % 4 == 0
    C4 = KT // 4               # exp chunks (4 key tiles each) per head
    TH = 2 * TS                # tokens per batch pair per core
    RG = [list(range(N_CORES))]

    nc = bacc.Bacc("TRN2", target_bir_lowering=False, debug=False,
                   num_devices=N_CORES)

    xT = nc.dram_tensor("xT", [C, TOK], BF, kind="ExternalInput").ap()
    wqT = nc.dram_tensor("wqT", [C, C], BF, kind="ExternalInput").ap()
    wkT = nc.dram_tensor("wkT", [C, C], BF, kind="ExternalInput").ap()
    wvT = nc.dram_tensor("wvT", [C, C], BF, kind="ExternalInput").ap()
    woT = nc.dram_tensor("woT", [C, C], BF, kind="ExternalInput").ap()
    bo_d = nc.dram_tensor("bo", [C, 1], F32, kind="ExternalInput").ap()
    out = nc.dram_tensor("out", [C, TOK], F32, kind="ExternalOutput").ap()

    with tile.TileContext(nc) as tc:
        with tc.tile_pool(name="dram", bufs=1, space="DRAM") as dram:
            # per batch-pair bounce + gathered buffers (K and V fused into
            # one AllGather payload per half)
            SZK = C * TH
            SZV = TH * HP5
            SZ = SZK + SZV
            kv_bn = [dram.tile([SZ], BF, name=f"kv_bn{i}") for i in range(2)]
            kv_all = [dram.tile([N_CORES * SZ], BF, addr_space="Shared",
                                name=f"kv_all{i}") for i in range(2)]
            k_bn = [t[0:SZK].rearrange("(r t) -> r t", t=TH) for t in kv_bn]
            v_bn = [t[SZK:SZ].rearrange("(p c) -> p c", c=HP5) for t in kv_bn]
            scr_d = dram.tile([128, 512], F32, name="scr_d")

            with tc.tile_pool(name="persist", bufs=1) as persist:
                qT_sb = persist.tile([128, CT, TOK], BF)
                attn_sb = persist.tile([128, CT, TOK], BF)
                ones_sb = persist.tile([128, 64], BF)
                nc.vector.memset(ones_sb[:], 1.0)
                wo_sb = persist.tile([128, CT, C], BF)
                bo_sb = persist.tile([128, CT, 1], F32)

                def proj_T(psum, w_sb, dram_dst, sbuf_dst, t_lo, t_hi, pool,
                           dst_ofs=0, psum_tag="mm", psum_bufs=6):
                    # out[o, t] = sum_i W^T[i, o] x^T[i, t]
                    for ot in range(CT):
                        for t0, tsz in _chunks(t_hi - t_lo, 512):
                            t0 += t_lo
                            ps = psum.tile([128, 512], F32, tag=psum_tag,
                                           bufs=psum_bufs, name="ps_proj")
                            for i in range(CT):
                                nc.tensor.matmul(
                                    ps[:, :tsz],
                                    w_sb[:, i, ot * 128:(ot + 1) * 128],
                                    xT_sb[:, i, t0:t0 + tsz],
                                    start=(i == 0), stop=(i == CT - 1))
                            if sbuf_dst is not None:
                                nc.vector.tensor_copy(
                                    sbuf_dst[:, ot, t0:t0 + tsz],
                                    ps[:, :tsz])
                            else:
                                st = pool.tile([128, 512], BF, tag="st",
                                               bufs=4, name="st_proj")
                                nc.vector.tensor_copy(st[:, :tsz],
                                                      ps[:, :tsz])
                                nc.sync.dma_start(
                                    dram_dst[ot * 128:(ot + 1) * 128,
                                             t0 - dst_ofs:
                                             t0 - dst_ofs + tsz],
                                    st[:, :tsz])

                def proj_V(psum, half, pool):
                    # V (token-major, per-head 65-padded with a ones col)
                    for ttl in range(TH // 128):
                        tt = half * (TH // 128) + ttl
                        stv = pool.tile([128, H, 65], BF, tag="stv", bufs=2,
                                        name="stv")
                        nc.vector.memset(stv[:, :, 64:65], 1.0)
                        for o0, osz in _chunks(C, 512):
                            ps = psum.tile([128, 512], F32, tag="mm",
                                           bufs=6, name="ps_v")
                            for i in range(CT):
                                nc.tensor.matmul(
                                    ps[:, :osz],
                                    xT_sb[:, i, tt * 128:(tt + 1) * 128],
                                    wv_sb[:, i, o0:o0 + osz],
                                    start=(i == 0), stop=(i == CT - 1))
                            h0, nh = o0 // 64, osz // 64
                            nc.vector.tensor_copy(
                                stv[:, h0:h0 + nh, 0:64],
                                ps[:, :osz].rearrange("p (h c) -> p h c",
                                                      c=64))
                        nc.sync.dma_start(
                            v_bn[half][ttl * 128:(ttl + 1) * 128, :],
                            stv[:].rearrange("p h c -> p (h c)"))

                # ---------------- Phase 1: K/V projections + AGs ----------
                with tc.tile_pool(name="p1", bufs=1) as p1, \
                     tc.tile_pool(name="psum1", bufs=1, space="PSUM") as psum1:
                    xT_sb = p1.tile([128, CT, TOK], BF)
                    nc.sync.dma_start(
                        xT_sb[:], xT.rearrange("(n p) t -> p n t", p=128))
                    wq_sb = p1.tile([128, CT, C], BF)
                    nc.sync.dma_start(
                        wq_sb[:], wqT.rearrange("(n p) o -> p n o", p=128))
                    wk_sb = p1.tile([128, CT, C], BF)
                    nc.sync.dma_start(
                        wk_sb[:], wkT.rearrange("(n p) o -> p n o", p=128))
                    wv_sb = p1.tile([128, CT, C], BF)
                    nc.sync.dma_start(
                        wv_sb[:], wvT.rearrange("(n p) o -> p n o", p=128))

                    for half in range(2):
                        proj_T(psum1, wk_sb, k_bn[half], None,
                               half * TH, (half + 1) * TH, p1,
                               dst_ofs=half * TH)
                        proj_V(psum1, half, p1)
                        nc.gpsimd.collective_compute(
                            "AllGather", mybir.AluOpType.bypass,
                            replica_groups=RG,
                            ins=[kv_bn[half][:].opt()],
                            outs=[kv_all[half][:].opt()])

                    # all of Q^T (overlaps the AllGathers)
                    proj_T(psum1, wq_sb, None, qT_sb, 0, TOK, p1)

                    nc.sync.dma_start(
                        wo_sb[:], woT.rearrange("(n p) o -> p n o", p=128))
                    nc.sync.dma_start(
                        bo_sb[:], bo_d.rearrange("(n p) o -> p n o", p=128))

                # ------------- Phase 2+3: attention + out-proj -------------
                with tc.tile_pool(name="p2", bufs=1) as p2, \
                     tc.tile_pool(name="psum2", bufs=1, space="PSUM") as psum2:
                    JR = TS // 128  # key tiles per rank chunk
                    # dedicated scratch bank: idempotent matmuls that keep
                    # the PE activity monitor from re-throttling the clock
                    # while ScalarE drains exp chunks
                    scr = psum2.tile([64, TS], F32, tag="scr", bufs=1,
                                     name="scr")

                    for b in range(B):
                        half, bb = b // 2, b % 2
                        kv_s = kv_all[half][:].rearrange(
                            "(s x) -> s x", s=N_CORES)
                        k_all_v = kv_s[:, 0:SZK].rearrange(
                            "s (r t) -> r s t", t=TH)
                        v_all_v = kv_s[:, SZK:SZ].rearrange(
                            "s (j p c) -> s j p c", p=128, c=HP5)

                        # V for this batch, two halves of key tiles
                        vbs = []
                        for kh in range(2):
                            vb = p2.tile([128, KT // 2, HP5], BF, tag="vb",
                                         bufs=2, name=f"vb{kh}")
                            for s0 in range(N_CORES // 2):
                                s = kh * (N_CORES // 2) + s0
                                nc.sync.dma_start(
                                    vb[:, s0 * JR:(s0 + 1) * JR, :],
                                    v_all_v[s, bb * JR:(bb + 1) * JR, :, :]
                                    .rearrange("j p c -> p j c"))
                            vbs.append(vb)

                        for hp in range(CT):
                            kp = p2.tile([128, N_CORES, TS], BF, tag="kp",
                                         bufs=2, name="kp")
                            nc.sync.dma_start(
                                kp[:],
                                k_all_v[hp * 128:(hp + 1) * 128, :,
                                        bb * TS:(bb + 1) * TS])
                            kp_f = kp[:].rearrange("p s t -> p (s t)")

                            P0 = p2.tile([128, KT * TS], BF, tag="P",
                                         bufs=2, name="P0")
                            P1 = p2.tile([128, KT * TS], BF, tag="P",
                                         bufs=2, name="P1")
                            pav0 = psum2.tile([65, TS], F32, tag="pav",
                                              bufs=2, name="pav0")
                            pav1 = psum2.tile([65, TS], F32, tag="pav",
                                              bufs=2, name="pav1")
                            pavs = (pav0, pav1)
                            for c4 in range(C4):
                                # separate tags -> stable separate banks per
                                # head so paired row-group matmuls can run
                                # concurrently on the PE
                                psA = psum2.tile([128, 4 * TS], F32,
                                                 tag="ps_a", bufs=1,
                                                 name="psA")
                                psB = psum2.tile([128, 4 * TS], F32,
                                                 tag="ps_b", bufs=1,
                                                 name="psB")
                                for j in range(4):
                                    kt = c4 * 4 + j
                                    for h, ps in ((0, psA), (1, psB)):
                                        nc.tensor.matmul(
                                            ps[:, j * TS:(j + 1) * TS],
                                            kp_f[h * 64:(h + 1) * 64,
                                                 kt * 128:(kt + 1) * 128],
                                            qT_sb[h * 64:(h + 1) * 64, hp,
                                                  b * TS:(b + 1) * TS],
                                            start=True, stop=True,
                                            tile_position=(h * 64, 0))
                                for ps, P in ((psA, P0), (psB, P1)):
                                    nc.scalar.activation(
                                        P[:, c4 * 4 * TS:(c4 + 1) * 4 * TS],
                                        ps[:],
                                        mybir.ActivationFunctionType.Exp,
                                        scale=SCALE)
                                for _ in range(2):
                                    nc.tensor.matmul(
                                        scr[:], ones_sb[:, 0:64],
                                        qT_sb[:, hp, b * TS:(b + 1) * TS],
                                        start=True, stop=True)
                                for h, P in ((0, P0), (1, P1)):
                                    hg = 2 * hp + h
                                    for j in range(4):
                                        kt = c4 * 4 + j
                                        vb = vbs[0] if kt < KT // 2 else vbs[1]
                                        ktl = kt % (KT // 2)
                                        nc.tensor.matmul(
                                            pavs[h][:],
                                            vb[:, ktl,
                                               hg * 65:(hg + 1) * 65],
                                            P[:, kt * TS:(kt + 1) * TS],
                                            start=(kt == 0),
                                            stop=(kt == KT - 1))

                            for h in (0, 1):
                                recip = p2.tile([128, TS], BF, tag="recip",
                                                bufs=2, name="recip")
                                with nc.allow_low_precision(
                                        reason="softmax denom in bf16"):
                                    nc.vector.reciprocal(recip[64:65, :],
                                                         pavs[h][64:65, :])
                                pbc = psum2.tile([64, TS], F32, tag="pbc",
                                                 bufs=1, name="pbc")
                                nc.tensor.matmul(
                                    pbc[:], ones_sb[64:65, :],
                                    recip[64:65, :],
                                    start=True, stop=True)
                                bcast_sb = p2.tile([64, TS], F32,
                                                   tag="bcast", bufs=2,
                                                   name="bcast_sb")
                                nc.vector.tensor_copy(bcast_sb[:], pbc[:])
                                tmp = p2.tile([64, TS], BF, tag="tmp",
                                              bufs=3, name="tmp")
                                nc.vector.tensor_mul(tmp[:], pavs[h][0:64, :],
                                                     bcast_sb[:])
                                nc.sync.dma_start(
                                    attn_sb[h * 64:(h + 1) * 64, hp,
                                            b * TS:(b + 1) * TS],
                                    tmp[:])

                        # out projection for this batch's tokens
                        for co in range(CT):
                            psy = psum2.tile([128, TS], F32, tag="ps_a",
                                             bufs=1, name="psy")
                            for ct in range(CT):
                                nc.tensor.matmul(
                                    psy[:],
                                    wo_sb[:, ct, co * 128:(co + 1) * 128],
                                    attn_sb[:, ct, b * TS:(b + 1) * TS],
                                    start=(ct == 0), stop=(ct == CT - 1))
                            ysb = p2.tile([128, TS], F32, tag="y", bufs=3,
                                          name="ysb")
                            nc.vector.tensor_scalar_add(
                                ysb[:], psy[:], bo_sb[:, co, :])
                            nc.sync.dma_start(
                                out[co * 128:(co + 1) * 128,
                                    b * TS:(b + 1) * TS],
                                ysb[:])

                    scr_sb = p2.tile([64, TS], F32, name="scr_sb")
                    nc.vector.tensor_copy(scr_sb[:], scr[:])
                    nc.sync.dma_start(scr_d[0:64, 0:TS], scr_sb[:])

    nc.compile()
    return nc


def _prep_inputs(hidden_states, Wq, Wk, Wv, Wo, bo):
    T = hidden_states.shape[1]
    TS = T // N_CORES
    TOK = B * TS
    bf = ml_dtypes.bfloat16
    wqT = np.ascontiguousarray(np.asarray(Wq, np.float32).T).astype(bf)
    wkT = np.ascontiguousarray(np.asarray(Wk, np.float32).T).astype(bf)
    wvT = np.ascontiguousarray(np.asarray(Wv, np.float32).T).astype(bf)
    woT = np.ascontiguousarray(np.asarray(Wo, np.float32).T).astype(bf)
    bo_c = np.asarray(bo, np.float32).reshape(C, 1)
    x = np.asarray(hidden_states, np.float32)
    in_maps = []
    for r in range(N_CORES):
        xr = x[:, r * TS:(r + 1) * TS, :].reshape(TOK, C)
        xTr = np.ascontiguousarray(xr.T).astype(bf)
        in_maps.append({
            "xT": xTr, "wqT": wqT, "wkT": wkT, "wvT": wvT, "woT": woT,
            "bo": bo_c,
        })
    return in_maps


def kernel(hidden_states, Wq, Wk, Wv, Wo, bo):
    global LAST_EXEC_TIME_NS
    _install_ntff_hook()
    Bx, T, Cx = hidden_states.shape
    assert (Bx, Cx) == (B, C)
    TS = T // N_CORES
    if T not in _BUILD_CACHE:
        _BUILD_CACHE[T] = build(T)
    nc = _BUILD_CACHE[T]
    in_maps = _prep_inputs(hidden_states, Wq, Wk, Wv, Wo, bo)
    res = run_bass_kernel_spmd(nc, in_maps, core_ids=list(range(N_CORES)))
    LAST_EXEC_TIME_NS = res.exec_time_ns
    outf = np.empty((B, T, C), np.float32)
    for r in range(N_CORES):
        yT = res.results[r]["out"]          # [C, TOK]
        yr = yT.T.reshape(B, TS, C)
        outf[:, r * TS:(r + 1) * TS, :] = yr
    return outf



# revision 2
# speedup vs baseline: 1.0282x; 1.0282x over previous
"""Distributed multi-head attention kernel for 8 Trainium2 NeuronCores.

Problem: y = softmax((x Wq^T)(x Wk^T)^T / sqrt(D)) (x Wv^T) Wo^T + bo
with B=4, T=2048, C=1280, H=20, D=64, float32 I/O.

Sharding (Megatron-style batch x head tensor parallel):
  Core r owns batch b = r//2 and heads [10*(r%2), 10*(r%2)+10).  Each
  core projects Q/K/V for its 10 heads from the full batch-b token
  stream (K/V/Q never leave SBUF -- no K/V AllGather at all), runs
  attention for all 2048 queries over its heads, applies its half of
  the output projection (row-split Wo), and ReduceScatters the partial
  [C, T] output with its pair core so each core ends up with the final
  activations for half the tokens.  The host reassembles batch/token
  tiles.

Attention runs in a transposed "S_T[k, q]" layout so the softmax
denominator falls out of the same matmul that computes P@V: V is
stored padded per head as [.., 65] with a constant-1 column, so psum
row 64 of the P@V accumulation is sum_k P.  exp() runs on ScalarE
directly out of PSUM in [128, 1024] chunks covering both heads of a
head-pair tile.

ScalarE is the roofline engine (1.2 GHz, ~0.37ms of exp work), so the
schedule exists to keep it saturated:

  - K / Q / output projections and the softmax-normalize broadcasts
    are queued as generator "filler groups" and dribbled into the PE
    stream a couple of matmuls per key tile, so ScalarE never waits
    more than one S_T matmul between exp chunks (this also keeps the
    PE HAM activity monitor from dropping the PE clock to 1.2 GHz);
  - filler groups accumulate in their own psum tag so their long-lived
    tiles never interleave with the S_T psum rotation (deadlock-free),
    and any group an upcoming S_T depends on is force-drained first;
  - query chunks are processed in order [0, 2, 1, 3] so the first
    ReduceScatter {qc0, qc2} and the first final-output pass run
    hidden under the last chunk's attention;
  - V projection is interleaved per key tile into the first chunk.

Compute dtype is bf16 (fp32 matmul is 4x slower on the PE array);
accumulation is fp32 in PSUM.  I/O stays fp32.
"""

import sys
import types
from collections import deque

import numpy as np
import ml_dtypes

import concourse.bass as bass
import concourse.mybir as mybir
import concourse.tile as tile
from concourse import bacc
from concourse.bass_utils import run_bass_kernel_spmd

N_CORES = 8
C = 1280          # model width
CL = 640          # local width (10 heads)
HL = 10           # local heads
HP = 5            # local head-pair tiles (128 rows = 2 heads x 64)
D = 64
B = 4
T = 2048          # full sequence length
TQ = 512          # query chunk
QC = T // TQ      # 4 query chunks
KT = T // 128     # 16 key tiles
CI = C // 128     # 10 contraction tiles
CO = C // 128     # 10 output tiles
CLT = CL // 128   # 5 local-channel tiles
PSL = 8           # P slab ring depth (key tiles resident)
BF = mybir.dt.bfloat16
F32 = mybir.dt.float32
SCALE = 1.0 / (D ** 0.5)

LAST_EXEC_TIME_NS = None
_BUILD_CACHE = {}


def _install_ntff_hook():
    """The trimmed antenv package lacks axon_hooks; register the NTFF
    profile hook by hand so trace=True can time the NEFF on silicon.
    Safe no-op if anything is missing."""
    if "antenv.axon_hooks" in sys.modules:
        return
    try:
        from trn_agent_boot.trn_boot import _ntff_profile_via_ctypes

        hook = _ntff_profile_via_ctypes("/opt/axon/libaxon_pjrt.so")
        mod = types.ModuleType("antenv.axon_hooks")
        mod.get_axon_ntff_profile_hook = lambda: hook
        mod.set_axon_ntff_profile_hook = lambda h: None
        sys.modules["antenv.axon_hooks"] = mod
        import antenv

        antenv.axon_hooks = mod
    except Exception:
        pass


def build():
    RG2 = [[2 * i, 2 * i + 1] for i in range(N_CORES // 2)]

    nc = bacc.Bacc("TRN2", target_bir_lowering=False, debug=False,
                   num_devices=N_CORES)

    xT = nc.dram_tensor("xT", [C, T], BF, kind="ExternalInput").ap()
    wqT = nc.dram_tensor("wqT", [C, CL], BF, kind="ExternalInput").ap()
    wkT = nc.dram_tensor("wkT", [C, CL], BF, kind="ExternalInput").ap()
    wvT = nc.dram_tensor("wvT", [C, CL], BF, kind="ExternalInput").ap()
    woT = nc.dram_tensor("woT", [CL, C], BF, kind="ExternalInput").ap()
    bo_d = nc.dram_tensor("bo", [C, 1], F32, kind="ExternalInput").ap()
    sel_d = nc.dram_tensor("sel", [HL, HP * 128], BF,
                           kind="ExternalInput").ap()
    out = nc.dram_tensor("out", [C, T // 2], F32, kind="ExternalOutput").ap()

    with tile.TileContext(nc) as tc:
        with tc.tile_pool(name="dram", bufs=1, space="DRAM") as dram:
            # bounce buffers for the pairwise ReduceScatter of the partial
            # output projection: j=0 carries {qc0, qc2}, j=1 {qc1, qc3}
            # (first half kept by the even rank, second by the odd rank).
            SZ = C * TQ
            y_bn = [dram.tile([2 * SZ], BF, name=f"y_bn{j}") for j in range(2)]
            y_rs = [dram.tile([SZ], BF, name=f"y_rs{j}") for j in range(2)]
            y_bn_v = [t[:].rearrange("(s o p t) -> s o p t", s=2, o=CO, p=128)
                      for t in y_bn]
            y_rs_v = [t[:].rearrange("(o p t) -> o p t", o=CO, p=128)
                      for t in y_rs]

            with tc.tile_pool(name="sb", bufs=1) as sb, \
                 tc.tile_pool(name="psum", bufs=1, space="PSUM") as psum:
                xT_sb = sb.tile([128, CI, T], BF)
                wq_sb = sb.tile([128, CI, CL], BF)
                wv_sb = sb.tile([128, CI, CL], BF)
                wo_sb = sb.tile([128, CLT, C], BF)
                bo_sb = sb.tile([128, CO, 1], F32)
                kT_sb = sb.tile([128, HP, T], BF)
                qT_sb = sb.tile([128, HP, T], BF)
                # token-major V, per-head 65-padded with a ones column
                vb = sb.tile([128, KT, HL * 65], BF)
                # staging for attention outputs of one qc (double buffered)
                attn_sb = [sb.tile([128, CLT, TQ], BF, name=f"attn{i}")
                           for i in range(2)]
                # staged P@V results ([64 dims | row 64 = denominator]) so
                # the psum banks recycle quickly
                pav_sb = {}
                for hp in range(HP):
                    for h in range(2):
                        pav_sb[(hp, h)] = sb.tile([65, TQ], BF,
                                                  name=f"pav_sb{hp}_{h}")
                den_sb = [sb.tile([HL, TQ], BF, name=f"den{i}")
                          for i in range(2)]
                den_f = sb.tile([HL, TQ], F32, name="den_f")
                rec_f = sb.tile([HL, TQ], F32, name="rec_f")
                rec_sb = [sb.tile([HL, TQ], BF, name=f"rec{i}")
                          for i in range(2)]
                # selector for broadcasting recip rows across partitions:
                # sel[p, hp, h, i] = 1 iff p == 2*hp + h, so
                # (sel[:, hp, h, :].T @ rec)[i, q] = rec[2*hp + h, q]
                sel_sb = sb.tile([HL, HP, 2, 64], BF, name="sel")
                nc.sync.dma_start(
                    sel_sb[:],
                    sel_d.rearrange("p (n h o) -> p n h o", h=2, o=64))
                xT_v = xT.rearrange("(n p) t -> p n t", p=128)
                for tc4 in range(QC):
                    nc.sync.dma_start(
                        xT_sb[:, :, tc4 * TQ:(tc4 + 1) * TQ],
                        xT_v[:, :, tc4 * TQ:(tc4 + 1) * TQ])
                nc.sync.dma_start(
                    wq_sb[:], wqT.rearrange("(n p) o -> p n o", p=128))
                nc.sync.dma_start(
                    wv_sb[:], wvT.rearrange("(n p) o -> p n o", p=128))
                nc.sync.dma_start(
                    wo_sb[:], woT.rearrange("(n p) o -> p n o", p=128))
                nc.sync.dma_start(
                    bo_sb[:], bo_d.rearrange("(n p) o -> p n o", p=128))

                vb_h = vb[:].rearrange("p k (h c) -> p k h c", c=65)
                nc.vector.memset(vb_h[:, :, :, 64:65], 1.0)
                wk_v = wkT.rearrange("(n p) o -> p n o", p=128)

                with tc.tile_pool(name="p2", bufs=1) as p2:
                    # exp(S_T) ring: PSL key tiles of [headA 512|headB 512];
                    # PV trails exp by <= 2 tiles so the ring never wraps
                    # onto live data
                    Pslab = p2.tile([128, PSL, 1024], BF)

                    def ps_tile(name):
                        return psum.tile([128, 1024], F32, tag="ps",
                                         bufs=2, name=name)

                    def fill_tile(name):
                        return psum.tile([128, TQ], F32, tag="fill",
                                         bufs=2, name=name)

                    def wk_slice(hp):
                        wk_sl = p2.tile([128, CI, 128], BF, tag="wk",
                                        bufs=2, name="wk_sl")
                        nc.sync.dma_start(
                            wk_sl[:], wk_v[:, :, hp * 128:(hp + 1) * 128])
                        return wk_sl

                    # ---- filler machinery: FIFO of (label, generator)
                    # groups emitting one PE matmul per step
                    queue = deque()

                    def step_filler(n):
                        done = 0
                        while done < n and queue:
                            try:
                                next(queue[0][1])
                                done += 1
                            except StopIteration:
                                queue.popleft()

                    def drain_label(label):
                        # run the queue until the labeled group completed
                        # (a group an upcoming matmul depends on must never
                        # stay queued behind it -- PE executes in order)
                        while any(g[0] == label for g in queue):
                            step_filler(16)

                    def drain_filler():
                        while queue:
                            step_filler(64)

                    def g_proj(w_sb, wi, dst_sb, ot, t0):
                        # dst[o, t] = sum_i W^T[i, o] x^T[i, t]
                        ps = fill_tile("ps_proj")
                        for i in range(CI):
                            nc.tensor.matmul(
                                ps[:],
                                w_sb[:, i, wi * 128:(wi + 1) * 128],
                                xT_sb[:, i, t0:t0 + TQ],
                                start=(i == 0), stop=(i == CI - 1))
                            yield
                        nc.vector.tensor_copy(dst_sb[:, ot, t0:t0 + TQ],
                                              ps[:])

                    def g_out_proj(pb, co, qc):
                        # partial y[co, tokens] = Wo_loc^T[:, co] @ attn
                        psy = fill_tile("psy")
                        for ct in range(CLT):
                            nc.tensor.matmul(
                                psy[:],
                                wo_sb[:, ct, co * 128:(co + 1) * 128],
                                attn_sb[pb][:, ct, :],
                                start=(ct == 0), stop=(ct == CLT - 1))
                            yield
                        ysb = p2.tile([128, TQ], BF, tag="ysb", bufs=2,
                                      name="ysb")
                        nc.vector.tensor_copy(ysb[:], psy[:])
                        nc.sync.dma_start(
                            y_bn_v[qc % 2][qc // 2, co, :, :], ysb[:])

                    def g_norm(pb, hp, h):
                        # broadcast 1/den over 64 partitions and normalize
                        pbc = fill_tile("pbc")
                        nc.tensor.matmul(
                            pbc[0:64, :],
                            sel_sb[:, hp, h, :],
                            rec_sb[pb][:],
                            start=True, stop=True)
                        yield
                        tmp = p2.tile([64, TQ], BF, tag="tmp", bufs=2,
                                      name="tmp")
                        nc.vector.tensor_mul(
                            tmp[:], pav_sb[(hp, h)][0:64, :], pbc[0:64, :])
                        nc.sync.dma_start(
                            attn_sb[pb][h * 64:(h + 1) * 64, hp, :], tmp[:])

                    def run_inline(gen):
                        for _ in gen:
                            pass

                    def recip_chain(pb):
                        nc.vector.tensor_copy(den_f[:], den_sb[pb][:])
                        nc.vector.reciprocal_approx_fast(rec_f[:], den_f[:])
                        nc.vector.tensor_copy(rec_sb[pb][:], rec_f[:])

                    def proj_v(tt, hp):
                        # V for token tile tt, head pair hp (token-major)
                        ps = ps_tile("ps_v")
                        for i in range(CI):
                            nc.tensor.matmul(
                                ps[:, :128],
                                xT_sb[:, i, tt * 128:(tt + 1) * 128],
                                wv_sb[:, i, hp * 128:(hp + 1) * 128],
                                start=(i == 0), stop=(i == CI - 1))
                        nc.vector.tensor_copy(
                            vb_h[:, tt, 2 * hp:2 * hp + 2, 0:64],
                            ps[:, :128].rearrange("p (h c) -> p h c", c=64))

                    def final_pass(j):
                        for co in range(CO):
                            yr = p2.tile([128, TQ], BF, tag="yrd", bufs=2,
                                         name="yr")
                            nc.sync.dma_start(yr[:], y_rs_v[j][co, :, :])
                            fo = p2.tile([128, TQ], F32, tag="fo", bufs=2,
                                         name="fo")
                            nc.vector.tensor_scalar_add(
                                fo[:], yr[:], bo_sb[:, co, :])
                            nc.sync.dma_start(
                                out[co * 128:(co + 1) * 128,
                                    j * TQ:(j + 1) * TQ],
                                fo[:])

                    def reduce_scatter(j):
                        nc.gpsimd.collective_compute(
                            "ReduceScatter", mybir.AluOpType.add,
                            replica_groups=RG2,
                            ins=[y_bn[j][:].opt()], outs=[y_rs[j][:].opt()])

                    # ---- ramp: K(hp0) + Q(qc0, ot0) (pipelined against the
                    # chunked x DMA); everything else is dribbled into the
                    # attention stream as filler.  A throwaway exp warms the
                    # ACT table so the ~2.7us load is off the critical path.
                    nc.scalar.activation(
                        rec_f[0:1, 0:1], den_f[0:1, 0:1],
                        mybir.ActivationFunctionType.Exp, scale=SCALE)
                    wk0 = wk_slice(0)
                    for tc4 in range(QC):
                        run_inline(g_proj(wk0, 0, kT_sb, 0, tc4 * TQ))
                    run_inline(g_proj(wq_sb, 0, qT_sb, 0, 0))

                    QCS = [0, 2, 1, 3]
                    for pidx, qc in enumerate(QCS):
                        pb = pidx % 2
                        first = pidx == 0
                        # queue: previous chunk's normalize + out-proj
                        # (before this chunk's pav copies overwrite pav_sb),
                        # this chunk's remaining Q, (first) remaining K
                        if first:
                            for hp in range(1, HP):
                                wk_sl = wk_slice(hp)
                                for tc4 in range(QC):
                                    queue.append((("k", hp), g_proj(
                                        wk_sl, 0, kT_sb, hp, tc4 * TQ)))
                                queue.append((("q", hp), g_proj(
                                    wq_sb, hp, qT_sb, hp, qc * TQ)))
                        else:
                            queue.append((("q", 1), g_proj(
                                wq_sb, 1, qT_sb, 1, qc * TQ)))
                            for hp in range(HP):
                                for h in (0, 1):
                                    queue.append((("n", hp, h),
                                                  g_norm(1 - pb, hp, h)))
                            for ot in range(2, HP):
                                queue.append((("q", ot), g_proj(
                                    wq_sb, ot, qT_sb, ot, qc * TQ)))
                            pqc = QCS[pidx - 1]
                            for co in range(CO):
                                queue.append((("o", co),
                                              g_out_proj(1 - pb, co, pqc)))

                        if pidx == 3:
                            reduce_scatter(0)

                        for hp in range(HP):
                            if hp:
                                drain_label(("k", hp))
                                drain_label(("q", hp))
                            elif not first:
                                drain_label(("q", 0))
                            pavs = [psum.tile([65, TQ], F32, tag="pav",
                                              bufs=2, name=f"pav{h}")
                                    for h in (0, 1)]

                            def s_t(kt):
                                # S_T[k in tile kt, q] for both heads
                                ps = ps_tile("ps_st")
                                for h in (0, 1):
                                    nc.tensor.matmul(
                                        ps[:, h * TQ:(h + 1) * TQ],
                                        kT_sb[h * 64:(h + 1) * 64, hp,
                                              kt * 128:(kt + 1) * 128],
                                        qT_sb[h * 64:(h + 1) * 64, hp,
                                              qc * TQ:(qc + 1) * TQ],
                                        start=True, stop=True,
                                        tile_position=(h * 64, 0))
                                nc.scalar.activation(
                                    Pslab[:, kt % PSL, :], ps[:],
                                    mybir.ActivationFunctionType.Exp,
                                    scale=SCALE)

                            def pv(kt):
                                for h in (0, 1):
                                    nc.tensor.matmul(
                                        pavs[h][:],
                                        vb_h[:, kt, 2 * hp + h, :],
                                        Pslab[:, kt % PSL,
                                              h * TQ:(h + 1) * TQ],
                                        start=(kt == 0), stop=(kt == KT - 1))

                            for kt in range(KT):
                                if first:
                                    proj_v(kt, hp)
                                s_t(kt)
                                if kt >= 1:
                                    pv(kt - 1)
                                step_filler(3 if first else 2)
                            pv(KT - 1)

                            # stage P@V to SBUF (psum recycles), gather the
                            # denominator rows into den_sb
                            for h in (0, 1):
                                if not first:
                                    # pav_sb write must not overtake the
                                    # queued normalize of the same tile
                                    drain_label(("n", hp, h))
                                nc.vector.tensor_copy(pav_sb[(hp, h)][:],
                                                      pavs[h][:])
                                nc.sync.dma_start(
                                    den_sb[pb][2 * hp + h:2 * hp + h + 1,
                                               :],
                                    pav_sb[(hp, h)][64:65, :])

                            if hp == HP - 1 and pidx < 3:
                                # next chunk's first Q projection
                                queue.append((("q", 0), g_proj(
                                    wq_sb, 0, qT_sb, 0,
                                    QCS[pidx + 1] * TQ)))

                        # 1/sum(exp) for this chunk (cheap, off critical
                        # path; the broadcasts run as filler next chunk)
                        recip_chain(pb)

                    # ---- tail: last chunk's normalize + output projection
                    # (final first half overlaps on DVE/DMA), then
                    # ReduceScatter {qc1, qc3} and the final second half
                    drain_filler()
                    for hp in range(HP):
                        for h in (0, 1):
                            run_inline(g_norm(1, hp, h))
                    for co in range(CO):
                        run_inline(g_out_proj(1, co, 3))
                    reduce_scatter(1)
                    final_pass(0)
                    final_pass(1)

    nc.compile()
    return nc


def _prep_inputs(hidden_states, Wq, Wk, Wv, Wo, bo):
    bf = ml_dtypes.bfloat16
    x = np.asarray(hidden_states, np.float32)
    Wq = np.asarray(Wq, np.float32)
    Wk = np.asarray(Wk, np.float32)
    Wv = np.asarray(Wv, np.float32)
    Wo = np.asarray(Wo, np.float32)
    bo_c = np.asarray(bo, np.float32).reshape(C, 1)
    sel = np.zeros((HL, HP, 2, 64), np.float32)
    for hp in range(HP):
        sel[2 * hp, hp, 0, :] = 1.0
        sel[2 * hp + 1, hp, 1, :] = 1.0
    sel = sel.reshape(HL, HP * 128).astype(bf)
    in_maps = []
    for r in range(N_CORES):
        b, h2 = r // 2, r % 2
        sl = slice(h2 * CL, (h2 + 1) * CL)
        xTr = np.ascontiguousarray(x[b].T).astype(bf)
        in_maps.append({
            "xT": xTr,
            "wqT": np.ascontiguousarray(Wq[sl, :].T).astype(bf),
            "wkT": np.ascontiguousarray(Wk[sl, :].T).astype(bf),
            "wvT": np.ascontiguousarray(Wv[sl, :].T).astype(bf),
            "woT": np.ascontiguousarray(Wo[:, sl].T).astype(bf),
            "bo": bo_c,
            "sel": sel,
        })
    return in_maps


def kernel(hidden_states, Wq, Wk, Wv, Wo, bo):
    global LAST_EXEC_TIME_NS
    _install_ntff_hook()
    Bx, Tx, Cx = hidden_states.shape
    assert (Bx, Tx, Cx) == (B, T, C)
    if "nc" not in _BUILD_CACHE:
        _BUILD_CACHE["nc"] = build()
    nc = _BUILD_CACHE["nc"]
    in_maps = _prep_inputs(hidden_states, Wq, Wk, Wv, Wo, bo)
    res = run_bass_kernel_spmd(nc, in_maps, core_ids=list(range(N_CORES)))
    LAST_EXEC_TIME_NS = res.exec_time_ns
    outf = np.empty((B, T, C), np.float32)
    for r in range(N_CORES):
        b, h2 = r // 2, r % 2
        yT = res.results[r]["out"]          # [C, T//2]
        outf[b, h2 * (T // 2):(h2 + 1) * (T // 2), :] = yT.T
    return outf
